# revision 38
# baseline (speedup 1.0000x reference)
"""Trainium2 Bass kernel for nn_BatchedFCN (batched ensemble MLP + max).

Reference computation (per network n of 1024, batch B=256):
    h = relu(x @ W1_n^T + b1); h = relu(h @ W2_n^T + b2); h = relu(h @ W3_n^T + b3)
    h = relu(h @ W4_n^T + b4); y_n = h @ W5_n^T + b5          # [B, 1]
    out[b] = max_n y_n[b]                                      # [B]

Sharding: the 1024 networks are split across 8 NeuronCores (128 nets/core).
Each core computes a partial max over its networks; the host folds the 8
partial results.

Per-core dataflow (activations transposed [features, batch]):
  L1 runs on the PE in fp8 DoubleRow perf mode (0.5 cycles/row). Accuracy is
  recovered with a 3-term hi/lo decomposition executed as 6 accumulating
  DoubleRow matmuls per net:
      16*h1 = Wh*xh + Wh*xl + Wl*xr
  where W=16*W1', Wh=e4m3(W), Wl=e4m3(16*(W-Wh)), xh=e4m3(x), xl=e4m3(x-xh),
  xr=e4m3(x/16).  The ACT evacuation applies relu with scale=1/16.
  L2/L3 are bf16 form-1 matmuls (weights stationary, activations moving).
  L4 folds |w5| and the w5 signs in: W4'' has 102 output columns
  [50 w5+-scaled | b5+ | 50 w5--scaled | b5-], and runs "form-2" (h3 slice
  stationary, W4'' moving, 102-wide) so h4 lands batch-major [128b, 102].
  L5 is then a DVE segmented sum over 51-column blocks plus a subtract:
      y = sum(block+) - sum(block-),
  written straight into a [128, 2*128] staging tile; a final reduce_max over
  pairs gives this core's per-batch max.

Weight/bias augmentation: one extra input row (bias) and one ones-column
propagate biases through every layer with no separate bias operands.
"""

import sys

import numpy as np

try:
    import concourse  # noqa: F401
except ImportError:  # fall back to the container's staged repo
    sys.path.insert(0, "/opt/trn_rl_repo")

import ml_dtypes  # noqa: E402

import concourse.mybir as mybir  # noqa: E402
import concourse.tile as tile  # noqa: E402
from concourse import bacc, bass_utils  # noqa: E402

# Problem shapes (hardcoded per contract)
NN = 1024  # total networks
B = 256  # batch
NCORES = 8
NPC = NN // NCORES  # networks per core = 128
PAIRS = NPC // 2  # 64
GROUPS = 4  # weight-DMA groups per core
GNETS = NPC // GROUPS  # 32 nets per group
GPAIRS = GNETS // 2  # 16 pairs per group

KA = 501  # augmented L1 contraction (500 inputs + bias row)
MA = 101  # augmented hidden width (100 + ones column)
MP = 112  # L1 per-k-tile output group, padded for dual-fp8 16B alignment
W1N = 4 * 2 * MP  # w1 cols per net = 896 (Wh_c0 | Wh_c1 | Wl_c0 | Wl_c1)
P0, P1 = 126, 125  # k-tile partition counts: c0 = feats 0..251, c1 = 252..500
M4C = 104  # L4'' cols per net: 51 +cols | 51 -cols | zsum-diff | pad (x0.5)
S1 = 16.0  # L1 fp8 weight scale

BF16 = ml_dtypes.bfloat16
E4M3 = ml_dtypes.float8_e4m3

_PROGRAM_CACHE = {}


def _build_program():
    nc = bacc.Bacc("TRN2", debug=False, num_devices=NCORES)
    f8 = mybir.dt.float8e4
    bf = mybir.dt.bfloat16
    f32 = mybir.dt.float32

    xp_d = nc.dram_tensor("xp", [128, 3072], f8, kind="ExternalInput").ap()
    w1_d = nc.dram_tensor("w1p", [128, NPC * W1N], f8, kind="ExternalInput").ap()
    w2_d = nc.dram_tensor("w2p", [MA, NPC * MA], bf, kind="ExternalInput").ap()
    w3_d = nc.dram_tensor("w3p", [MA, NPC * MA], bf, kind="ExternalInput").ap()
    w4_d = nc.dram_tensor("w4p", [MA, NPC * M4C], bf, kind="ExternalInput").ap()
    out_d = nc.dram_tensor("out", [128, 2], f32, kind="ExternalOutput").ap()

    relu = mybir.ActivationFunctionType.Relu
    DR = mybir.MatmulPerfMode.DoubleRow

    # L1 matmul schedule: (k-partitions, w col offset within net, x col offset)
    L1MM = [
        (P0, 0, 0),  # Wh_c0 x xh_c0
        (P1, 2 * MP, 512),  # Wh_c1 x xh_c1
        (P0, 0, 1024),  # Wh_c0 x xl_c0
        (P1, 2 * MP, 1536),  # Wh_c1 x xl_c1
        (P0, 4 * MP, 2048),  # Wl_c0 x xr_c0
        (P1, 6 * MP, 2560),  # Wl_c1 x xr_c1
    ]

    def r3(ap):
        return ap.rearrange("p (two m) -> p two m", two=2)

    with tile.TileContext(nc) as tc:
        from contextlib import ExitStack

        with ExitStack() as ctx:
            consts = ctx.enter_context(tc.tile_pool(name="consts", bufs=1))
            wp1 = ctx.enter_context(tc.tile_pool(name="wp1", bufs=2))
            wp2 = ctx.enter_context(tc.tile_pool(name="wp2", bufs=2))
            wp3 = ctx.enter_context(tc.tile_pool(name="wp3", bufs=2))
            wp4 = ctx.enter_context(tc.tile_pool(name="wp4", bufs=2))
            hp = ctx.enter_context(tc.tile_pool(name="hp", bufs=3))
            # PSUM: p1 [128,512]x2 (2 banks) + shared p2/p3 quad [128,1024]x2
            # (4 banks) + p4 [128,408]x2 (2 banks) = 8 banks exactly
            pp = ctx.enter_context(tc.tile_pool(name="pp", bufs=2, space="PSUM"))

            # x goes on the ACT HWDGE ring so it doesn't queue behind the
            # first w1 chunk on the SP ring
            xp = consts.tile([128, 3072], f8)
            nc.scalar.dma_start(xp, xp_d)
            # y staging: col = half*128 + 2*pair + netloc, written once per pair
            y_all = consts.tile([128, 256], f32)
            # trigger the one-time ACT table load while the first weight DMA
            # is still in flight
            warm = consts.tile([1, 2], f32)
            nc.vector.memset(warm, 0.0)
            nc.scalar.activation(warm[0:1, 1:2], warm[0:1, 0:1], relu)

            # Software pipeline over pairs p (quads q = p//2):
            #   L1@p  act1@p+1  L2@p+2  relu2f(q)@2q+3  L3@p+4  act3f(q)@2q+5
            #   L4@p+6  relu4@p+7  reducef+subf(q)@2q+8
            # Quad fusion halves the per-instruction access-latency tax on the
            # ACT/DVE evacuations of L2/L3/L4.
            SKEW_MAX = 8
            DMA_LEAD = 14
            group_tiles = {}
            p1_t, p2_t, p3_t, p4_t = {}, {}, {}, {}
            h1_t, h2_t, h3_t, h4_t = {}, {}, {}, {}
            for t in range(PAIRS + SKEW_MAX):
                # weight DMAs, prefetched DMA_LEAD steps ahead of first use
                tl = t + DMA_LEAD
                if t == 0 or (tl % GPAIRS == 0 and tl // GPAIRS < GROUPS):
                    g = 0 if t == 0 else tl // GPAIRS
                    w1t = wp1.tile([128, GNETS * W1N], f8, tag="w1")
                    w2t = wp2.tile([MA, GNETS * MA], bf, tag="w2")
                    w3t = wp3.tile([MA, GNETS * MA], bf, tag="w3")
                    w4t = wp4.tile([MA, GNETS * M4C], bf, tag="w4")

                    def chunk(wt, wd, nw, n0, n1, g=g):
                        nc.sync.dma_start(
                            wt[:, n0 * nw : n1 * nw],
                            wd[:, (g * GNETS + n0) * nw : (g * GNETS + n1) * nw],
                        )

                    def w1_chunk(n0, n1):
                        chunk(w1t, w1_d, W1N, n0, n1)

                    if g == 0:
                        # deadline-ordered: w1 nets n are needed at step n/2,
                        # w2 at +2, w3 at +4, w4 at +6
                        w1_chunk(0, 2)
                        w1_chunk(2, 4)
                        chunk(w2t, w2_d, MA, 0, 32)
                        w1_chunk(4, 8)
                        chunk(w3t, w3_d, MA, 0, 16)
                        w1_chunk(8, 14)
                        chunk(w4t, w4_d, M4C, 0, 16)
                        w1_chunk(14, 18)
                        w1_chunk(18, 23)
                        chunk(w3t, w3_d, MA, 16, 32)
                        w1_chunk(23, 28)
                        chunk(w4t, w4_d, M4C, 16, 32)
                        w1_chunk(28, 32)
                    else:
                        w1_chunk(0, 2)
                        w1_chunk(2, 4)
                        chunk(w2t, w2_d, MA, 0, GNETS)
                        w1_chunk(4, 8)
                        chunk(w3t, w3_d, MA, 0, GNETS)
                        w1_chunk(8, 14)
                        chunk(w4t, w4_d, M4C, 0, GNETS)
                        w1_chunk(14, 23)
                        w1_chunk(23, 32)
                    group_tiles[g] = (w1t, w2t, w3t, w4t)

                def loc(p):
                    # group-local A/B net indices for pair p
                    jj = p % GPAIRS
                    return p // GPAIRS, 2 * jj, 2 * jj + 1

                # ---- PE stage L2 (pair t-2): into quad tile half
                p_ = t - 2
                if 0 <= p_ < PAIRS:
                    g, nA, nB_ = loc(p_)
                    w2t = group_tiles[g][1]
                    h1 = h1_t.pop(p_)
                    p2 = pp.tile([128, 512], f32, tag="pmid", name="p2p", bufs=4)
                    p2_t[p_] = p2
                    for nl, fo in ((nA, 0), (nB_, B)):
                        nc.tensor.matmul(
                            p2[0:MA, fo : fo + B],
                            lhsT=w2t[:, nl * MA : (nl + 1) * MA],
                            rhs=h1[:, fo : fo + B],
                        )

                # ---- PE stage L3 (pair t-4): into quad tile half
                p_ = t - 4
                if 0 <= p_ < PAIRS:
                    g, nA, nB_ = loc(p_)
                    w3t = group_tiles[g][2]
                    h2 = h2_t.pop(p_)
                    p3 = pp.tile([128, 512], f32, tag="pmid", name="p3p", bufs=4)
                    p3_t[p_] = p3
                    for nl, fo in ((nA, 0), (nB_, B)):
                        nc.tensor.matmul(
                            p3[0:MA, fo : fo + B],
                            lhsT=w3t[:, nl * MA : (nl + 1) * MA],
                            rhs=h2[:, fo : fo + B],
                        )

                # ---- PE stage L4'' (pair t-6): form-2, h3 stationary
                p_ = t - 6
                if 0 <= p_ < PAIRS:
                    g, nA, nB_ = loc(p_)
                    w4t = group_tiles[g][3]
                    h3 = h3_t.pop(p_)
                    p4 = pp.tile([128, 4 * M4C], f32, tag="p4")
                    # p4 col layout: (2*h + j)*M4C
                    for j, nl in enumerate((nA, nB_)):
                        for h in range(2):
                            nc.tensor.matmul(
                                p4[0:128, (2 * h + j) * M4C : (2 * h + j + 1) * M4C],
                                lhsT=h3[
                                    0:MA, j * B + h * 128 : j * B + (h + 1) * 128
                                ],
                                rhs=w4t[:, nl * M4C : (nl + 1) * M4C],
                            )
                    p4_t[p_] = p4

                # ---- PE stage L1 (pair t): 6 DoubleRow matmuls per net
                p_ = t
                if 0 <= p_ < PAIRS:
                    g, nA, nB_ = loc(p_)
                    w1t = group_tiles[g][0]
                    p1 = pp.tile([128, 512], f32, tag="p1")
                    for nl, fo in ((nA, 0), (nB_, B)):
                        for j, (P, wo, xo) in enumerate(L1MM):
                            nc.tensor.matmul(
                                p1[0:MP, fo : fo + B],
                                lhsT=r3(
                                    w1t[0:P, nl * W1N + wo : nl * W1N + wo + 2 * MP]
                                ),
                                rhs=r3(xp[0:P, xo : xo + 512]),
                                start=(j == 0),
                                stop=(j == len(L1MM) - 1),
                                perf_mode=DR,
                            )
                    p1_t[p_] = p1

                # ---- ACT stage act1 (pair t-1): h1 = relu(p1/16)
                p_ = t - 1
                if 0 <= p_ < PAIRS:
                    p1 = p1_t.pop(p_)
                    h1 = hp.tile([MA, 512], bf, tag="h1")
                    nc.scalar.activation(h1, p1[0:MA, :], relu, scale=1.0 / S1)
                    h1_t[p_] = h1

                # ---- ACT stage act3 (pair t-5)
                p_ = t - 5
                if 0 <= p_ < PAIRS:
                    p3 = p3_t.pop(p_)
                    h3 = hp.tile([MA, 512], bf, tag="h3")
                    nc.scalar.activation(h3, p3[0:MA, :], relu)
                    h3_t[p_] = h3

                # ---- relu2 (pair t-3): DVE, with every 8th pair on ACT
                # to equalize the two evacuation engines' load
                p_ = t - 3
                if 0 <= p_ < PAIRS:
                    p2 = p2_t.pop(p_)
                    h2 = hp.tile([MA, 512], bf, tag="h2")
                    if p_ % 24 == 0:
                        nc.scalar.activation(h2, p2[0:MA, :], relu)
                    else:
                        nc.vector.tensor_scalar_max(h2, p2[0:MA, :], 0.0)
                    h2_t[p_] = h2

                # ---- L5 via relu(z) = (z+|z|)/2 (pair t-7):
                # DVE abs-reduce on PSUM, Pool sign-subtract, DVE combine
                # with the PE-computed linear column.
                p_ = t - 7
                if 0 <= p_ < PAIRS:
                    p4 = p4_t.pop(p_)
                    p4v = p4.rearrange("p (blk c) -> p blk c", blk=4)
                    r2 = hp.tile([128, 8], bf, tag="r2")
                    with nc.allow_low_precision("51-term bf16 sums, ~2^-8 rel"):
                        nc.vector.tensor_reduce(
                            r2,
                            p4v[:, :, 0:102].rearrange(
                                "p blk (s i) -> p blk s i", s=2
                            ),
                            axis=mybir.AxisListType.X,
                            op=mybir.AluOpType.add,
                            apply_absolute_value=True,
                        )
                    r2v = r2.rearrange("p (blk s) -> p blk s", s=2)
                    rd = hp.tile([128, 4], bf, tag="rd")
                    nc.gpsimd.tensor_tensor(
                        rd, r2v[:, :, 0], r2v[:, :, 1], mybir.AluOpType.subtract
                    )
                    # y = zsum-col + (|S|+ - |S|-); blk = 2*half + netloc
                    yo = y_all.rearrange("p (h q) -> p h q", h=2)
                    nc.vector.tensor_tensor(
                        yo[:, :, 2 * p_ : 2 * p_ + 2],
                        p4v[:, :, 102].rearrange("p (h j) -> p h j", h=2),
                        rd.rearrange("p (h j) -> p h j", h=2),
                        mybir.AluOpType.add,
                    )

            # final per-core fold: max over the 128 pair-net columns per half
            m_t = hp.tile([128, 2], f32, tag="m")
            nc.vector.tensor_reduce(
                m_t,
                y_all.rearrange("p (h q) -> p h q", h=2),
                axis=mybir.AxisListType.X,
                op=mybir.AluOpType.max,
            )
            nc.sync.dma_start(out_d, m_t)

    nc.compile()
    return nc


def _get_program():
    if "nc" not in _PROGRAM_CACHE:
        _PROGRAM_CACHE["nc"] = _build_program()
    return _PROGRAM_CACHE["nc"]


def _q8(a):
    return a.astype(E4M3).astype(np.float32)


def _pack_inputs(inputs):
    """Host-side: transpose, augment, hi/lo-fp8 decompose L1, shard."""
    x = np.asarray(inputs["x"], np.float32)
    w = {i: np.asarray(inputs[f"w{i}"], np.float32) for i in (1, 2, 3, 4, 5)}
    b = {i: np.asarray(inputs[f"b{i}"], np.float32) for i in (1, 2, 3, 4, 5)}

    # ---- L1 operands: augmented xT' = [x^T ; ones] [501, 256]
    xT = np.concatenate([x.T, np.ones((1, B), np.float32)], axis=0)
    xh = _q8(xT)
    xl = _q8(xT - xh)
    xr = _q8(xT / S1)

    def pack_x(xv, base, P):
        # -> [128, 512]: (p, i*256+n) = xv[base + i*P + p, n]
        o = np.zeros((128, 2, B), np.float32)
        nf = min(2 * P, KA - base)
        v = np.zeros((2 * P, B), np.float32)
        v[:nf] = xv[base : base + nf]
        o[0:P] = v.reshape(2, P, B).transpose(1, 0, 2)
        return o.reshape(128, 2 * B)

    xp = np.concatenate(
        [
            pack_x(xh, 0, P0), pack_x(xh, 252, P1),
            pack_x(xl, 0, P0), pack_x(xl, 252, P1),
            pack_x(xr, 0, P0), pack_x(xr, 252, P1),
        ],
        axis=1,
    ).astype(E4M3)  # [128, 3072]

    # ---- W1': [N, 501, 101] scaled by 16, hi/lo e4m3
    W1 = np.zeros((NN, KA, MA), np.float32)
    W1[:, :500, :100] = w[1].transpose(0, 2, 1)
    W1[:, 500, :100] = b[1]
    W1[:, 500, 100] = 1.0
    W1 *= S1
    Wh = _q8(W1)
    Wl = _q8((W1 - Wh) * 16.0)

    def pack_w(Wv, base, P):
        # -> [128, N, 2, MP]: (p, n, i, m) = Wv[n, base + i*P + p, m]
        o = np.zeros((128, NN, 2, MP), np.float32)
        nf = min(2 * P, KA - base)
        v = np.zeros((NN, 2 * P, MA), np.float32)
        v[:, :nf] = Wv[:, base : base + nf]
        o[0:P, :, :, 0:MA] = v.reshape(NN, 2, P, MA).transpose(2, 0, 1, 3)
        return o

    # per net: [4 groups][2 k-tiles][MP]: Wh_c0 | Wh_c1 | Wl_c0 | Wl_c1
    w1p = np.stack(
        [pack_w(Wh, 0, P0), pack_w(Wh, 252, P1), pack_w(Wl, 0, P0), pack_w(Wl, 252, P1)],
        axis=2,
    ).astype(E4M3)  # [128, N, 4, 2, MP]

    def aug_mid(wi, bi):
        # -> [101(part=i), N, 101]; ones-propagation col + bias row folded in
        A = np.zeros((NN, MA, MA), np.float32)
        A[:, :100, :100] = wi.transpose(0, 2, 1)
        A[:, 100, :100] = bi
        A[:, 100, 100] = 1.0
        return A.transpose(1, 0, 2).astype(BF16)

    w2p = aug_mid(w[2], b[2])  # [101, N, 101]
    w3p = aug_mid(w[3], b[3])

    # ---- W4'': |w5| folded in, sign-split, b5 columns; form-2 moving operand
    w5v = w[5][:, 0, :]  # [N, 50]
    w5p = np.maximum(w5v, 0.0) * 0.5
    w5n = np.maximum(-w5v, 0.0) * 0.5
    b5v = b[5][:, 0]
    A = np.zeros((NN, M4C, MA), np.float32)  # [N, out col, feature]
    A[:, 0:50, :100] = w[4] * w5p[:, :, None]
    A[:, 0:50, 100] = b[4] * w5p
    A[:, 50, 100] = np.maximum(b5v, 0.0) * 0.5
    A[:, 51:101, :100] = w[4] * w5n[:, :, None]
    A[:, 51:101, 100] = b[4] * w5n
    A[:, 101, 100] = np.maximum(-b5v, 0.0) * 0.5
    # col 102 computes the linear part of y on the PE:
    #   y = sum(relu(z)) - sum(relu(zneg)) = (S+ - S-) + (|S|+ - |S|-)
    # with z halved; col 102 = sum(+cols) - sum(-cols)
    A[:, 102, :] = A[:, 0:51, :].sum(axis=1) - A[:, 51:102, :].sum(axis=1)
    w4p = A.transpose(2, 0, 1).astype(BF16)  # [101, N, 104]

    in_maps = []
    for c in range(NCORES):
        sl = slice(c * NPC, (c + 1) * NPC)
        in_maps.append(
            {
                "xp": xp,
                "w1p": np.ascontiguousarray(
                    w1p[:, sl].reshape(128, NPC * W1N)
                ),
                "w2p": np.ascontiguousarray(w2p[:, sl].reshape(MA, NPC * MA)),
                "w3p": np.ascontiguousarray(w3p[:, sl].reshape(MA, NPC * MA)),
                "w4p": np.ascontiguousarray(w4p[:, sl].reshape(MA, NPC * M4C)),
            }
        )
    return in_maps


def _fold_outputs(results):
    r = np.stack([np.asarray(res["out"], np.float32) for res in results])  # [8,128,2]
    m = r.max(axis=0)  # [128, 2]
    return np.ascontiguousarray(m.T.reshape(B)).astype(np.float32)


def run(inputs, **run_kwargs):
    """Pack, execute on 8 cores, fold. Returns (output[B], BassKernelResults)."""
    nc = _get_program()
    in_maps = _pack_inputs(inputs)
    res = bass_utils.run_bass_kernel_spmd(
        nc, in_maps, core_ids=list(range(NCORES)), **run_kwargs
    )
    return _fold_outputs(res.results), res


def kernel(**inputs):
    out, _ = run(inputs)
    return out


# revision 39
# speedup vs baseline: 1.0105x; 1.0105x over previous
"""Trainium2 Bass kernel for nn_BatchedFCN (batched ensemble MLP + max).

Reference computation (per network n of 1024, batch B=256):
    h = relu(x @ W1_n^T + b1); h = relu(h @ W2_n^T + b2); h = relu(h @ W3_n^T + b3)
    h = relu(h @ W4_n^T + b4); y_n = h @ W5_n^T + b5          # [B, 1]
    out[b] = max_n y_n[b]                                      # [B]

Sharding: the 1024 networks are split across 8 NeuronCores (128 nets/core).
Each core computes a partial max over its networks; the host folds the 8
partial results.

Per-core dataflow (activations transposed [features, batch]):
  L1 runs on the PE in fp8 DoubleRow perf mode (0.5 cycles/row). Accuracy is
  recovered with a 3-term hi/lo decomposition executed as 6 accumulating
  DoubleRow matmuls per net:
      16*h1 = Wh*xh + Wh*xl + Wl*xr
  where W=16*W1', Wh=e4m3(W), Wl=e4m3(16*(W-Wh)), xh=e4m3(x), xl=e4m3(x-xh),
  xr=e4m3(x/16).  The ACT evacuation applies relu with scale=1/16.
  L2/L3 are bf16 form-1 matmuls (weights stationary, activations moving).
  L4 folds |w5| and the w5 signs in: W4'' has 102 output columns
  [50 w5+-scaled | b5+ | 50 w5--scaled | b5-], and runs "form-2" (h3 slice
  stationary, W4'' moving, 102-wide) so h4 lands batch-major [128b, 102].
  L5 is then a DVE segmented sum over 51-column blocks plus a subtract:
      y = sum(block+) - sum(block-),
  written straight into a [128, 2*128] staging tile; a final reduce_max over
  pairs gives this core's per-batch max.

Weight/bias augmentation: one extra input row (bias) and one ones-column
propagate biases through every layer with no separate bias operands.
"""

import sys

import numpy as np

try:
    import concourse  # noqa: F401
except ImportError:  # fall back to the container's staged repo
    sys.path.insert(0, "/opt/trn_rl_repo")

import ml_dtypes  # noqa: E402

import concourse.mybir as mybir  # noqa: E402
import concourse.tile as tile  # noqa: E402
from concourse import bacc, bass_utils  # noqa: E402

# Problem shapes (hardcoded per contract)
NN = 1024  # total networks
B = 256  # batch
NCORES = 8
NPC = NN // NCORES  # networks per core = 128
PAIRS = NPC // 2  # 64
GROUPS = 4  # weight-DMA groups per core
GNETS = NPC // GROUPS  # 32 nets per group
GPAIRS = GNETS // 2  # 16 pairs per group

KA = 501  # augmented L1 contraction (500 inputs + bias row)
MA = 101  # augmented hidden width (100 + ones column)
MP = 112  # L1 per-k-tile output group, padded for dual-fp8 16B alignment
W1N = 4 * 2 * MP  # w1 cols per net = 896 (Wh_c0 | Wh_c1 | Wl_c0 | Wl_c1)
P0, P1 = 126, 125  # k-tile partition counts: c0 = feats 0..251, c1 = 252..500
M4C = 104  # L4'' cols per net: 51 +cols | 51 -cols | zsum-diff | pad (x0.5)
S1 = 16.0  # L1 fp8 weight scale

BF16 = ml_dtypes.bfloat16
E4M3 = ml_dtypes.float8_e4m3

_PROGRAM_CACHE = {}


def _build_program():
    nc = bacc.Bacc("TRN2", debug=False, num_devices=NCORES)
    f8 = mybir.dt.float8e4
    bf = mybir.dt.bfloat16
    f32 = mybir.dt.float32

    xp_d = nc.dram_tensor("xp", [128, 3072], f8, kind="ExternalInput").ap()
    w1_d = nc.dram_tensor("w1p", [128, NPC * W1N], f8, kind="ExternalInput").ap()
    w2_d = nc.dram_tensor("w2p", [MA, NPC * MA], bf, kind="ExternalInput").ap()
    w3_d = nc.dram_tensor("w3p", [MA, NPC * MA], bf, kind="ExternalInput").ap()
    w4_d = nc.dram_tensor("w4p", [MA, NPC * M4C], bf, kind="ExternalInput").ap()
    out_d = nc.dram_tensor("out", [128, 2], f32, kind="ExternalOutput").ap()

    relu = mybir.ActivationFunctionType.Relu
    DR = mybir.MatmulPerfMode.DoubleRow

    # L1 matmul schedule: (k-partitions, w col offset within net, x col offset)
    L1MM = [
        (P0, 0, 0),  # Wh_c0 x xh_c0
        (P1, 2 * MP, 512),  # Wh_c1 x xh_c1
        (P0, 0, 1024),  # Wh_c0 x xl_c0
        (P1, 2 * MP, 1536),  # Wh_c1 x xl_c1
        (P0, 4 * MP, 2048),  # Wl_c0 x xr_c0
        (P1, 6 * MP, 2560),  # Wl_c1 x xr_c1
    ]

    def r3(ap):
        return ap.rearrange("p (two m) -> p two m", two=2)

    with tile.TileContext(nc) as tc:
        from contextlib import ExitStack

        with ExitStack() as ctx:
            consts = ctx.enter_context(tc.tile_pool(name="consts", bufs=1))
            wp1 = ctx.enter_context(tc.tile_pool(name="wp1", bufs=2))
            wp2 = ctx.enter_context(tc.tile_pool(name="wp2", bufs=2))
            wp3 = ctx.enter_context(tc.tile_pool(name="wp3", bufs=2))
            wp4 = ctx.enter_context(tc.tile_pool(name="wp4", bufs=2))
            hp = ctx.enter_context(tc.tile_pool(name="hp", bufs=3))
            # PSUM: p1 [128,512]x2 (2 banks) + shared p2/p3 quad [128,1024]x2
            # (4 banks) + p4 [128,408]x2 (2 banks) = 8 banks exactly
            pp = ctx.enter_context(tc.tile_pool(name="pp", bufs=2, space="PSUM"))

            # x goes on the ACT HWDGE ring so it doesn't queue behind the
            # first w1 chunk on the SP ring
            xp = consts.tile([128, 3072], f8)
            nc.scalar.dma_start(xp, xp_d)
            # y staging: col = half*128 + 2*pair + netloc, written once per pair
            y_all = consts.tile([128, 256], f32)
            # trigger the one-time ACT table load while the first weight DMA
            # is still in flight
            warm = consts.tile([1, 2], f32)
            nc.vector.memset(warm, 0.0)
            nc.scalar.activation(warm[0:1, 1:2], warm[0:1, 0:1], relu)

            # Software pipeline over pairs p (quads q = p//2):
            #   L1@p  act1@p+1  L2@p+2  relu2f(q)@2q+3  L3@p+4  act3f(q)@2q+5
            #   L4@p+6  relu4@p+7  reducef+subf(q)@2q+8
            # Quad fusion halves the per-instruction access-latency tax on the
            # ACT/DVE evacuations of L2/L3/L4.
            SKEW_MAX = 8
            DMA_LEAD = 14
            group_tiles = {}
            p1_t, p2_t, p3_t, p4_t = {}, {}, {}, {}
            h1_t, h2_t, h3_t, h4_t = {}, {}, {}, {}
            for t in range(PAIRS + SKEW_MAX):
                # weight DMAs, prefetched DMA_LEAD steps ahead of first use
                tl = t + DMA_LEAD
                if t == 0 or (tl % GPAIRS == 0 and tl // GPAIRS < GROUPS):
                    g = 0 if t == 0 else tl // GPAIRS
                    w1t = wp1.tile([128, GNETS * W1N], f8, tag="w1")
                    w2t = wp2.tile([MA, GNETS * MA], bf, tag="w2")
                    w3t = wp3.tile([MA, GNETS * MA], bf, tag="w3")
                    w4t = wp4.tile([MA, GNETS * M4C], bf, tag="w4")

                    def chunk(wt, wd, nw, n0, n1, g=g):
                        nc.sync.dma_start(
                            wt[:, n0 * nw : n1 * nw],
                            wd[:, (g * GNETS + n0) * nw : (g * GNETS + n1) * nw],
                        )

                    def w1_chunk(n0, n1):
                        chunk(w1t, w1_d, W1N, n0, n1)

                    if g == 0:
                        # deadline-ordered: w1 nets n are needed at step n/2,
                        # w2 at +2, w3 at +4, w4 at +6
                        w1_chunk(0, 2)
                        w1_chunk(2, 4)
                        chunk(w2t, w2_d, MA, 0, 32)
                        w1_chunk(4, 8)
                        chunk(w3t, w3_d, MA, 0, 16)
                        w1_chunk(8, 14)
                        chunk(w4t, w4_d, M4C, 0, 16)
                        w1_chunk(14, 18)
                        w1_chunk(18, 23)
                        chunk(w3t, w3_d, MA, 16, 32)
                        w1_chunk(23, 28)
                        chunk(w4t, w4_d, M4C, 16, 32)
                        w1_chunk(28, 32)
                    else:
                        w1_chunk(0, 2)
                        w1_chunk(2, 4)
                        chunk(w2t, w2_d, MA, 0, GNETS)
                        w1_chunk(4, 8)
                        chunk(w3t, w3_d, MA, 0, GNETS)
                        w1_chunk(8, 14)
                        chunk(w4t, w4_d, M4C, 0, GNETS)
                        w1_chunk(14, 23)
                        w1_chunk(23, 32)
                    group_tiles[g] = (w1t, w2t, w3t, w4t)

                def loc(p):
                    # group-local A/B net indices for pair p
                    jj = p % GPAIRS
                    return p // GPAIRS, 2 * jj, 2 * jj + 1

                # ---- PE stage L2 (pair t-2): into quad tile half
                p_ = t - 2
                if 0 <= p_ < PAIRS:
                    g, nA, nB_ = loc(p_)
                    w2t = group_tiles[g][1]
                    h1 = h1_t.pop(p_)
                    p2 = pp.tile([128, 512], f32, tag="pmid", name="p2p", bufs=4)
                    p2_t[p_] = p2
                    for nl, fo in ((nA, 0), (nB_, B)):
                        nc.tensor.matmul(
                            p2[0:MA, fo : fo + B],
                            lhsT=w2t[:, nl * MA : (nl + 1) * MA],
                            rhs=h1[:, fo : fo + B],
                        )

                # ---- PE stage L3 (pair t-4): into quad tile half
                p_ = t - 4
                if 0 <= p_ < PAIRS:
                    g, nA, nB_ = loc(p_)
                    w3t = group_tiles[g][2]
                    h2 = h2_t.pop(p_)
                    p3 = pp.tile([128, 512], f32, tag="pmid", name="p3p", bufs=4)
                    p3_t[p_] = p3
                    for nl, fo in ((nA, 0), (nB_, B)):
                        nc.tensor.matmul(
                            p3[0:MA, fo : fo + B],
                            lhsT=w3t[:, nl * MA : (nl + 1) * MA],
                            rhs=h2[:, fo : fo + B],
                        )

                # ---- PE stage L4'' (pair t-6): form-2, h3 stationary
                p_ = t - 6
                if 0 <= p_ < PAIRS:
                    g, nA, nB_ = loc(p_)
                    w4t = group_tiles[g][3]
                    h3 = h3_t.pop(p_)
                    p4 = pp.tile([128, 4 * M4C], f32, tag="p4")
                    # p4 col layout: (2*h + j)*M4C
                    for j, nl in enumerate((nA, nB_)):
                        for h in range(2):
                            nc.tensor.matmul(
                                p4[0:128, (2 * h + j) * M4C : (2 * h + j + 1) * M4C],
                                lhsT=h3[
                                    0:MA, j * B + h * 128 : j * B + (h + 1) * 128
                                ],
                                rhs=w4t[:, nl * M4C : (nl + 1) * M4C],
                            )
                    p4_t[p_] = p4

                # ---- PE stage L1 (pair t): 6 DoubleRow matmuls per net
                p_ = t
                if 0 <= p_ < PAIRS:
                    g, nA, nB_ = loc(p_)
                    w1t = group_tiles[g][0]
                    p1 = pp.tile([128, 512], f32, tag="p1")
                    for nl, fo in ((nA, 0), (nB_, B)):
                        for j, (P, wo, xo) in enumerate(L1MM):
                            nc.tensor.matmul(
                                p1[0:MP, fo : fo + B],
                                lhsT=r3(
                                    w1t[0:P, nl * W1N + wo : nl * W1N + wo + 2 * MP]
                                ),
                                rhs=r3(xp[0:P, xo : xo + 512]),
                                start=(j == 0),
                                stop=(j == len(L1MM) - 1),
                                perf_mode=DR,
                            )
                    p1_t[p_] = p1

                # ---- ACT stage act1 (pair t-1): h1 = relu(p1/16)
                p_ = t - 1
                if 0 <= p_ < PAIRS:
                    p1 = p1_t.pop(p_)
                    h1 = hp.tile([MA, 512], bf, tag="h1")
                    nc.scalar.activation(h1, p1[0:MA, :], relu, scale=1.0 / S1)
                    h1_t[p_] = h1

                # ---- ACT stage act3 (pair t-5)
                p_ = t - 5
                if 0 <= p_ < PAIRS:
                    p3 = p3_t.pop(p_)
                    h3 = hp.tile([MA, 512], bf, tag="h3")
                    nc.scalar.activation(h3, p3[0:MA, :], relu)
                    h3_t[p_] = h3

                # ---- relu2 (pair t-3): DVE, with every 8th pair on ACT
                # to equalize the two evacuation engines' load
                p_ = t - 3
                if 0 <= p_ < PAIRS:
                    p2 = p2_t.pop(p_)
                    h2 = hp.tile([MA, 512], bf, tag="h2")
                    if p_ % 16 == 0:
                        nc.scalar.activation(h2, p2[0:MA, :], relu)
                    else:
                        nc.vector.tensor_scalar_max(h2, p2[0:MA, :], 0.0)
                    h2_t[p_] = h2

                # ---- L5 via relu(z) = (z+|z|)/2 (pair t-7):
                # DVE abs-reduce on PSUM, Pool sign-subtract, DVE combine
                # with the PE-computed linear column.
                p_ = t - 7
                if 0 <= p_ < PAIRS:
                    p4 = p4_t.pop(p_)
                    p4v = p4.rearrange("p (blk c) -> p blk c", blk=4)
                    r2 = hp.tile([128, 8], bf, tag="r2")
                    with nc.allow_low_precision("51-term bf16 sums, ~2^-8 rel"):
                        nc.vector.tensor_reduce(
                            r2,
                            p4v[:, :, 0:102].rearrange(
                                "p blk (s i) -> p blk s i", s=2
                            ),
                            axis=mybir.AxisListType.X,
                            op=mybir.AluOpType.add,
                            apply_absolute_value=True,
                        )
                    r2v = r2.rearrange("p (blk s) -> p blk s", s=2)
                    rd = hp.tile([128, 4], bf, tag="rd")
                    nc.gpsimd.tensor_tensor(
                        rd, r2v[:, :, 0], r2v[:, :, 1], mybir.AluOpType.subtract
                    )
                    # y = zsum-col + (|S|+ - |S|-); blk = 2*half + netloc
                    yo = y_all.rearrange("p (h q) -> p h q", h=2)
                    nc.vector.tensor_tensor(
                        yo[:, :, 2 * p_ : 2 * p_ + 2],
                        p4v[:, :, 102].rearrange("p (h j) -> p h j", h=2),
                        rd.rearrange("p (h j) -> p h j", h=2),
                        mybir.AluOpType.add,
                    )

            # final per-core fold: max over the 128 pair-net columns per half
            m_t = hp.tile([128, 2], f32, tag="m")
            nc.vector.tensor_reduce(
                m_t,
                y_all.rearrange("p (h q) -> p h q", h=2),
                axis=mybir.AxisListType.X,
                op=mybir.AluOpType.max,
            )
            nc.sync.dma_start(out_d, m_t)

    nc.compile()
    return nc


def _get_program():
    if "nc" not in _PROGRAM_CACHE:
        _PROGRAM_CACHE["nc"] = _build_program()
    return _PROGRAM_CACHE["nc"]


def _q8(a):
    return a.astype(E4M3).astype(np.float32)


def _pack_inputs(inputs):
    """Host-side: transpose, augment, hi/lo-fp8 decompose L1, shard."""
    x = np.asarray(inputs["x"], np.float32)
    w = {i: np.asarray(inputs[f"w{i}"], np.float32) for i in (1, 2, 3, 4, 5)}
    b = {i: np.asarray(inputs[f"b{i}"], np.float32) for i in (1, 2, 3, 4, 5)}

    # ---- L1 operands: augmented xT' = [x^T ; ones] [501, 256]
    xT = np.concatenate([x.T, np.ones((1, B), np.float32)], axis=0)
    xh = _q8(xT)
    xl = _q8(xT - xh)
    xr = _q8(xT / S1)

    def pack_x(xv, base, P):
        # -> [128, 512]: (p, i*256+n) = xv[base + i*P + p, n]
        o = np.zeros((128, 2, B), np.float32)
        nf = min(2 * P, KA - base)
        v = np.zeros((2 * P, B), np.float32)
        v[:nf] = xv[base : base + nf]
        o[0:P] = v.reshape(2, P, B).transpose(1, 0, 2)
        return o.reshape(128, 2 * B)

    xp = np.concatenate(
        [
            pack_x(xh, 0, P0), pack_x(xh, 252, P1),
            pack_x(xl, 0, P0), pack_x(xl, 252, P1),
            pack_x(xr, 0, P0), pack_x(xr, 252, P1),
        ],
        axis=1,
    ).astype(E4M3)  # [128, 3072]

    # ---- W1': [N, 501, 101] scaled by 16, hi/lo e4m3
    W1 = np.zeros((NN, KA, MA), np.float32)
    W1[:, :500, :100] = w[1].transpose(0, 2, 1)
    W1[:, 500, :100] = b[1]
    W1[:, 500, 100] = 1.0
    W1 *= S1
    Wh = _q8(W1)
    Wl = _q8((W1 - Wh) * 16.0)

    def pack_w(Wv, base, P):
        # -> [128, N, 2, MP]: (p, n, i, m) = Wv[n, base + i*P + p, m]
        o = np.zeros((128, NN, 2, MP), np.float32)
        nf = min(2 * P, KA - base)
        v = np.zeros((NN, 2 * P, MA), np.float32)
        v[:, :nf] = Wv[:, base : base + nf]
        o[0:P, :, :, 0:MA] = v.reshape(NN, 2, P, MA).transpose(2, 0, 1, 3)
        return o

    # per net: [4 groups][2 k-tiles][MP]: Wh_c0 | Wh_c1 | Wl_c0 | Wl_c1
    w1p = np.stack(
        [pack_w(Wh, 0, P0), pack_w(Wh, 252, P1), pack_w(Wl, 0, P0), pack_w(Wl, 252, P1)],
        axis=2,
    ).astype(E4M3)  # [128, N, 4, 2, MP]

    def aug_mid(wi, bi):
        # -> [101(part=i), N, 101]; ones-propagation col + bias row folded in
        A = np.zeros((NN, MA, MA), np.float32)
        A[:, :100, :100] = wi.transpose(0, 2, 1)
        A[:, 100, :100] = bi
        A[:, 100, 100] = 1.0
        return A.transpose(1, 0, 2).astype(BF16)

    w2p = aug_mid(w[2], b[2])  # [101, N, 101]
    w3p = aug_mid(w[3], b[3])

    # ---- W4'': |w5| folded in, sign-split, b5 columns; form-2 moving operand
    w5v = w[5][:, 0, :]  # [N, 50]
    w5p = np.maximum(w5v, 0.0) * 0.5
    w5n = np.maximum(-w5v, 0.0) * 0.5
    b5v = b[5][:, 0]
    A = np.zeros((NN, M4C, MA), np.float32)  # [N, out col, feature]
    A[:, 0:50, :100] = w[4] * w5p[:, :, None]
    A[:, 0:50, 100] = b[4] * w5p
    A[:, 50, 100] = np.maximum(b5v, 0.0) * 0.5
    A[:, 51:101, :100] = w[4] * w5n[:, :, None]
    A[:, 51:101, 100] = b[4] * w5n
    A[:, 101, 100] = np.maximum(-b5v, 0.0) * 0.5
    # col 102 computes the linear part of y on the PE:
    #   y = sum(relu(z)) - sum(relu(zneg)) = (S+ - S-) + (|S|+ - |S|-)
    # with z halved; col 102 = sum(+cols) - sum(-cols)
    A[:, 102, :] = A[:, 0:51, :].sum(axis=1) - A[:, 51:102, :].sum(axis=1)
    w4p = A.transpose(2, 0, 1).astype(BF16)  # [101, N, 104]

    in_maps = []
    for c in range(NCORES):
        sl = slice(c * NPC, (c + 1) * NPC)
        in_maps.append(
            {
                "xp": xp,
                "w1p": np.ascontiguousarray(
                    w1p[:, sl].reshape(128, NPC * W1N)
                ),
                "w2p": np.ascontiguousarray(w2p[:, sl].reshape(MA, NPC * MA)),
                "w3p": np.ascontiguousarray(w3p[:, sl].reshape(MA, NPC * MA)),
                "w4p": np.ascontiguousarray(w4p[:, sl].reshape(MA, NPC * M4C)),
            }
        )
    return in_maps


def _fold_outputs(results):
    r = np.stack([np.asarray(res["out"], np.float32) for res in results])  # [8,128,2]
    m = r.max(axis=0)  # [128, 2]
    return np.ascontiguousarray(m.T.reshape(B)).astype(np.float32)


def run(inputs, **run_kwargs):
    """Pack, execute on 8 cores, fold. Returns (output[B], BassKernelResults)."""
    nc = _get_program()
    in_maps = _pack_inputs(inputs)
    res = bass_utils.run_bass_kernel_spmd(
        nc, in_maps, core_ids=list(range(NCORES)), **run_kwargs
    )
    return _fold_outputs(res.results), res


def kernel(**inputs):
    out, _ = run(inputs)
    return out
